# revision 58
# baseline (speedup 1.0000x reference)
"""GIN discriminator (4-layer GINConv + global mean pool + sigmoid) on 8 trn2 cores.

Sharding: nodes are split contiguously across 8 cores (6250 each). Each layer:
  - activations of all nodes are replicated per-core in DRAM (bf16), via AllGather
  - each core gathers edge-source rows for edges whose dst it owns (dma_gather),
    scatter-adds them per 128-dst tile with one-hot matmuls into PSUM (+ identity
    matmul adds x_own), transposes h to feature-major on the PE, and runs the
    spectral-normalized MLP in fp32.
Pooling: per-core partial graph sums via one-hot matmul, AllReduce, then
counts/fc/sigmoid replicated on every core. Spectral norm of the weights and all
edge bucketing run on the host in numpy.
"""

import numpy as np
import ml_dtypes

import concourse.bass as bass
import concourse.bacc as bacc
import concourse.mybir as mybir
import concourse.tile as tile
from concourse.bass_utils import run_bass_kernel_spmd

BF16 = mybir.dt.bfloat16
F32 = mybir.dt.float32
I16 = mybir.dt.int16
nbf16 = ml_dtypes.bfloat16

# ---------------- problem config (hardcoded for the graded problem) ----------
CORES = 8
N = 50000
E = 800000
G = 64
D_IN = 128
H = 512
N_LAYERS = 4
SN_ITERS = 5

P = 128          # partitions


def _bank_geometry(npc, tiles):
    """Tile-aligned bank splits (per-rank row ranges) for the split AllGather.

    Three banks at full size: A hides under mid-layer compute, B1 under the
    tail, B2 is the small exposed remainder."""
    if tiles >= 12:
        tsplits = [tiles // 2, tiles - max(2, tiles // 12), tiles]
    elif tiles >= 2:
        tsplits = [(tiles + 1) // 2, tiles]
    else:
        tsplits = [tiles]
    starts = [0] + [min(t * P, npc) for t in tsplits]
    return [(starts[i], starts[i + 1]) for i in range(len(tsplits))]


NPC = N // CORES                      # nodes per core
TILES = -(-NPC // P)                  # dst tiles per core
LAST_ROWS = NPC - (TILES - 1) * P     # rows in the last tile
NCHUNKS = -(-NPC // 512)              # node chunks (512 nodes) per core
BANKS = _bank_geometry(NPC, TILES)    # [(row_start, row_end) per rank]
NBANKS = len(BANKS)


def cdiv(a, b):
    return -(-a // b)


def _no_cc():
    import os

    return os.environ.get("KBASS_NO_CC", "0") == "1"


import os as _os

MAX_GATHER_CHUNKS = int(_os.environ.get("KBASS_MAXCH", "8"))
N_SWDGE_QUEUES = int(_os.environ.get("KBASS_NSWQ", "4"))
SWDGE_SCRATCH = int(_os.environ.get("KBASS_SCRATCH", "16384"))


def _patch_tile_swdge_lanes():
    """Partition Tile's 8 DMASW completion-sem lanes by SWDGE queue (2 lanes
    per queue) instead of global round-robin. With multiple SWDGE queues, the
    default round-robin can put DMAs from different queues on one lane, which
    breaks the per-lane FIFO-completion invariant Tile's sync model assumes
    (the simulator rejects it as a queue/sem lock violation)."""
    import concourse.tile_sem_assignment as tsa
    from concourse.tile_scheduler import DMAInst

    if getattr(tsa.TileClockTick, "_kbass_qaware", False):
        return
    orig = tsa.TileClockTick._assign_tick

    def _assign_tick(self, inst):
        if (
            isinstance(inst, DMAInst)
            and inst.engine == mybir.EngineType.Pool
            and not isinstance(inst, bass_isa.UserSyncedRemoteDMADescs)
        ):
            q = getattr(inst, "queue_num", 0) or 0
            lanes_per_q = max(1, self.swdge_sem_count // N_SWDGE_QUEUES)
            if not hasattr(self, "_kbass_qtog"):
                self._kbass_qtog = {}
            tog = self._kbass_qtog.get(q, 0)
            self._kbass_qtog[q] = (tog + 1) % lanes_per_q
            self.next_sw_dma_idx = (q * lanes_per_q + tog) % self.swdge_sem_count
        return orig(self, inst)

    tsa.TileClockTick._assign_tick = _assign_tick
    tsa.TileClockTick._kbass_qaware = True


def configure(n=50000, e=800000, g=64, d_in=128, h=512, n_layers=4):
    """Reconfigure module geometry (used by test harnesses for small smoke runs)."""
    global N, E, G, D_IN, H, N_LAYERS, NPC, TILES, LAST_ROWS, NCHUNKS
    global BANKS, NBANKS
    N, E, G, D_IN, H, N_LAYERS = n, e, g, d_in, h, n_layers
    NPC = N // CORES
    TILES = -(-NPC // P)
    LAST_ROWS = NPC - (TILES - 1) * P
    NCHUNKS = -(-NPC // 512)
    BANKS = _bank_geometry(NPC, TILES)
    NBANKS = len(BANKS)
    _prog_cache.clear()


def tiles_of_chunk(c):
    return list(range(4 * c, min(4 * c + 4, TILES)))


def tile_rows(t):
    return LAST_ROWS if t == TILES - 1 else P


# ---------------- host-side math ---------------------------------------------
def _spectral_normalize(W):
    W = np.asarray(W, np.float32)
    u = np.ones((W.shape[0],), np.float32) / np.float32(np.sqrt(np.float32(W.shape[0])))
    for _ in range(SN_ITERS):
        v = W.T @ u
        v = v / (np.linalg.norm(v) + np.float32(1e-12))
        u = W @ v
        u = u / (np.linalg.norm(u) + np.float32(1e-12))
    sigma = u @ (W @ v)
    return (W / sigma).astype(np.float32)


def _pack_call(idx, n_chunks):
    """int16 idxs for one dma_gather call: index i lives at [i%16, i//16],
    replicated across the eight 16-partition groups (one per Q7 core)."""
    L = np.zeros((n_chunks * P,), np.int16)
    L[: len(idx)] = idx.astype(np.int16)
    return np.tile(L.reshape(-1, 16).T, (8, 1))  # [128, n_chunks*8]


def _preprocess_edges(edge_index, x0bf):
    """Bucket edges by (dst core, dst tile, src bank); uniform chunk counts.

    Bank mapping (matches the split AllGather): global node g with r=g//NPC,
    i=g%NPC goes to bank A row r*BSPLIT+i if i<BSPLIT else bank B row
    r*(NPC-BSPLIT)+(i-BSPLIT).

    Also builds the layer-1 pre-gathered edge features (x0bf[src] in chunk
    order) so layer 1 needs no on-device gather.
    """
    src = np.asarray(edge_index[0], np.int64)
    dst = np.asarray(edge_index[1], np.int64)
    core = dst // NPC
    tloc = (dst % NPC) // P
    dloc = (dst % NPC) % P
    r = src // NPC
    i = src % NPC
    bstarts = np.array([b[0] for b in BANKS] + [NPC], np.int64)
    bank = np.searchsorted(bstarts, i, side="right") - 1
    brows = bstarts[1:] - bstarts[:-1]
    srcloc = r * brows[bank] + (i - bstarts[bank])

    key = (core * TILES + tloc) * NBANKS + bank
    order = np.argsort(key, kind="stable")
    key_s, srcloc_s, dloc_s, src_s = key[order], srcloc[order], dloc[order], src[order]
    counts = np.bincount(key_s, minlength=CORES * TILES * NBANKS).reshape(
        CORES, TILES, NBANKS
    )
    starts = np.zeros(CORES * TILES * NBANKS + 1, np.int64)
    np.cumsum(counts.reshape(-1), out=starts[1:])

    # uniform (max over cores) chunk counts per tile/bank
    nch = np.maximum(cdiv(counts.max(axis=0), P), 1)  # [TILES, NBANKS]
    ncht = nch.sum(axis=1)                            # [TILES]
    tot_ch = int(ncht.sum())
    idx_cols = tot_ch * 8
    dloc_cols = tot_ch

    idx16 = np.zeros((CORES, P, idx_cols), np.int16)
    dlocs = np.full((CORES, P, dloc_cols), -1.0, nbf16)
    x1g = np.zeros((CORES, tot_ch * P, D_IN), nbf16)
    for c in range(CORES):
        icol = 0
        dcol = 0
        for t in range(TILES):
            for b in range(NBANKS):
                k = (c * TILES + t) * NBANKS + b
                s, e = starts[k], starts[k + 1]
                nchb = int(nch[t, b])
                idx16[c, :, icol : icol + nchb * 8] = _pack_call(srcloc_s[s:e], nchb)
                dl = np.full((nchb * P,), -1.0, np.float32)
                dl[: e - s] = dloc_s[s:e]
                dlocs[c, :, dcol : dcol + nchb] = (
                    dl.reshape(nchb, P).T.astype(nbf16)
                )
                gsrc = np.zeros((nchb * P,), np.int64)
                gsrc[: e - s] = src_s[s:e]
                x1g[c, dcol * P : (dcol + nchb) * P, :] = x0bf[gsrc]
                icol += nchb * 8
                dcol += nchb
    return nch, idx16, dlocs, x1g


def _build_pool_onehot(batch):
    batch = np.asarray(batch, np.int64)
    pool = np.zeros((CORES, P, TILES * G), np.float32)
    for c in range(CORES):
        b = batch[c * NPC : (c + 1) * NPC]
        for i in range(NPC):
            t, p = i // P, i % P
            pool[c, p, t * G + int(b[i])] = 1.0
    counts = np.bincount(batch, minlength=G).astype(np.float32)
    cinv = (1.0 / np.maximum(counts, 1.0)).astype(np.float32)
    return pool, cinv


# ---------------- device program ---------------------------------------------
from concourse import bass_isa


def build_program(nch):
    _patch_tile_swdge_lanes()
    nch = np.asarray(nch)
    ncht = nch.sum(axis=1)
    nch_max = int(ncht.max())
    idx_cols = int(ncht.sum()) * 8
    dloc_cols = int(ncht.sum())

    nc = bacc.Bacc(
        num_devices=CORES,
        target_bir_lowering=False,
        debug=False,
        num_swdge_queues=N_SWDGE_QUEUES,
        dynamic_dma_scratch_size=SWDGE_SCRATCH,
    )

    tot_ch = int(ncht.sum())

    # ---- external inputs
    x1g = nc.declare_dram_parameter("x1g", [tot_ch * P, D_IN], BF16, isOutput=False)
    xown0 = nc.declare_dram_parameter("xown0", [NPC, D_IN], BF16, isOutput=False)
    idx16 = nc.declare_dram_parameter("idx16", [P, idx_cols], I16, isOutput=False)
    dlocs = nc.declare_dram_parameter("dlocs", [P, dloc_cols], BF16, isOutput=False)
    pool1h = nc.declare_dram_parameter("pool1h", [P, TILES * G], BF16, isOutput=False)
    w1t0 = nc.declare_dram_parameter("w1t0", [D_IN, H], F32, isOutput=False)
    w1tr = nc.declare_dram_parameter("w1tr", [(N_LAYERS - 1) * H, H], F32, isOutput=False)
    w2t = nc.declare_dram_parameter("w2t", [N_LAYERS * H, H], F32, isOutput=False)
    b1c = nc.declare_dram_parameter("b1c", [P, N_LAYERS * 4], F32, isOutput=False)
    b2bc = nc.declare_dram_parameter("b2bc", [N_LAYERS * P, H], F32, isOutput=False)
    iotar = nc.declare_dram_parameter("iotar", [P, nch_max * P], BF16, isOutput=False)
    ident16 = nc.declare_dram_parameter("ident16", [P, P], BF16, isOutput=False)
    identf = nc.declare_dram_parameter("identf", [P, P], F32, isOutput=False)
    cinv = nc.declare_dram_parameter("cinv", [G, 1], F32, isOutput=False)
    fcwb = nc.declare_dram_parameter("fcwb", [G, H], F32, isOutput=False)
    fcb = nc.declare_dram_parameter("fcb", [G, 1], F32, isOutput=False)
    out_ext = nc.declare_dram_parameter("out", [G, 1], F32, isOutput=True)

    # ---- internal DRAM (double-buffered per layer parity)
    agx = [
        [
            nc.dram_tensor(f"ag{b}_{i}", [BANKS[b][1] - BANKS[b][0], H], BF16)
            for b in range(NBANKS)
        ]
        for i in range(2)
    ]
    xfx = [
        [
            nc.dram_tensor(
                f"xf{b}_{i}",
                [CORES * (BANKS[b][1] - BANKS[b][0]), H],
                BF16,
                addr_space="Shared",
            )
            for b in range(NBANKS)
        ]
        for i in range(2)
    ]
    prb = nc.dram_tensor("prb", [G, H], F32)
    pro = nc.dram_tensor("pro", [G, H], F32, addr_space="Shared")

    rg = [list(range(CORES))]

    with tile.TileContext(nc) as tc:
        with (
            tc.tile_pool(name="consts", bufs=1) as cpool,
            tc.tile_pool(name="wts", bufs=1) as wpool,
            tc.tile_pool(name="edge", bufs=8) as epool,
            tc.tile_pool(name="bsel", bufs=3) as bpool,
            tc.tile_pool(name="xo", bufs=4) as xopool,
            tc.tile_pool(name="hsb", bufs=5) as hpool,
            tc.tile_pool(name="hfm", bufs=2) as fpool,
            tc.tile_pool(name="zt", bufs=6) as zpool,
            tc.tile_pool(name="agt", bufs=2) as agpool,
            tc.tile_pool(name="ps_agg", bufs=2, space="PSUM") as agg_ps,
            tc.tile_pool(name="ps_tp", bufs=1, space="PSUM") as tp_ps,
            tc.tile_pool(name="ps_z", bufs=2, space="PSUM") as z_ps,
            tc.tile_pool(name="ps_h2", bufs=2, space="PSUM") as h2_ps,
            tc.tile_pool(name="ps_pool", bufs=1, space="PSUM") as pool_ps,
        ):
            # ---- load constants
            idx_sb = cpool.tile([P, idx_cols], I16)
            nc.sync.dma_start(idx_sb[:], idx16[:, :])
            dloc_sb = cpool.tile([P, dloc_cols], BF16)
            nc.sync.dma_start(dloc_sb[:], dlocs[:, :])
            iota_sb = cpool.tile([P, nch_max * P], BF16)
            nc.sync.dma_start(iota_sb[:], iotar[:, :])
            id16_sb = cpool.tile([P, P], BF16)
            nc.sync.dma_start(id16_sb[:], ident16[:, :])
            idf_sb = cpool.tile([P, P], F32)
            nc.sync.dma_start(idf_sb[:], identf[:, :])
            b1_sb = cpool.tile([P, N_LAYERS * 4], F32)
            nc.sync.dma_start(b1_sb[:], b1c[:, :])
            cinv_sb = cpool.tile([G, 1], F32)
            nc.sync.dma_start(cinv_sb[:], cinv[:, :])
            fcw_sb = cpool.tile([G, H], F32)
            nc.sync.dma_start(fcw_sb[:], fcwb[:, :])
            fcb_sb = cpool.tile([G, 1], F32)
            nc.sync.dma_start(fcb_sb[:], fcb[:, :])
            pool_sb = cpool.tile([P, TILES * G], BF16)
            nc.sync.dma_start(pool_sb[:], pool1h[:, :])

            self_qn = [0]  # rotating SWDGE queue assignment for gathers
            for lay in range(N_LAYERS):
                din = D_IN if lay == 0 else H
                fch = din // P  # feature chunks of the layer input
                if lay == 0:
                    banks = None
                    xo_src = None
                else:
                    banks = [t_[:, :] for t_ in xfx[(lay - 1) % 2]]
                    xo_src = agx[(lay - 1) % 2]

                # per-layer weights
                w1t_sb = wpool.tile([P, fch * H], F32, tag="w1t")
                if lay == 0:
                    nc.sync.dma_start(w1t_sb[:, 0:H], w1t0[:, :])
                else:
                    for fi in range(fch):
                        nc.sync.dma_start(
                            w1t_sb[:, fi * H : (fi + 1) * H],
                            w1tr[(lay - 1) * H + fi * P : (lay - 1) * H + (fi + 1) * P, :],
                        )
                w2t_sb = wpool.tile([P, 4 * H], F32, tag="w2t")
                for zf in range(4):
                    nc.sync.dma_start(
                        w2t_sb[:, zf * H : (zf + 1) * H],
                        w2t[lay * H + zf * P : lay * H + (zf + 1) * P, :],
                    )
                b2_sb = wpool.tile([P, H], F32, tag="b2")
                nc.sync.dma_start(b2_sb[:], b2bc[lay * P : (lay + 1) * P, :])

                if lay == N_LAYERS - 1:
                    poolps = pool_ps.tile([G, H], F32)

                for c in range(NCHUNKS):
                    tlist = tiles_of_chunk(c)
                    nodes_c = sum(tile_rows(t) for t in tlist)
                    # -- phase 1: issue gathers + one-hot gen for every tile of
                    # the chunk (per-call edge tiles: matmuls start as soon as
                    # each call lands, and DMA prefetch runs well ahead of PE)
                    pre = {}
                    for t in tlist:
                        rows = tile_rows(t)
                        xo = xopool.tile([P, din], BF16, tag="xo")
                        if rows < P:
                            nc.vector.memset(xo[:], 0.0)
                        if lay == 0:
                            nc.sync.dma_start(
                                xo[:rows, :], xown0[t * P : t * P + rows, :]
                            )
                        else:
                            bt = next(
                                bi for bi, (s0, e0) in enumerate(BANKS)
                                if s0 <= t * P < e0
                            )
                            o = t * P - BANKS[bt][0]
                            nc.sync.dma_start(
                                xo[:rows, :], xo_src[bt][o : o + rows, :]
                            )
                        ncht_t = int(ncht[t])
                        icol = int(ncht[:t].sum()) * 8
                        dcol = int(ncht[:t].sum())
                        calls = []  # (etile, n_chunks_in_call)
                        if lay == 0:
                            et = epool.tile([P, ncht_t * din], BF16, tag="etile")
                            nc.sync.dma_start(
                                et[:, :].rearrange("p (k j) -> p k j", j=din),
                                x1g[dcol * P : (dcol + ncht_t) * P, :].rearrange(
                                    "(k p) j -> p k j", p=P
                                ),
                            )
                            calls.append((et, ncht_t))
                        else:
                            for b in range(NBANKS):
                                nchb = int(nch[t, b])
                                done = 0
                                while done < nchb:
                                    nsub = min(MAX_GATHER_CHUNKS, nchb - done)
                                    nidx = nsub * P
                                    et = epool.tile(
                                        [P, MAX_GATHER_CHUNKS * din], BF16,
                                        tag="etile",
                                    )
                                    nc.gpsimd.dma_gather(
                                        out_ap=et[:, 0 : nsub * din].rearrange(
                                            "p (s e) -> p s e", e=din
                                        ),
                                        in_ap=banks[b],
                                        idxs_ap=idx_sb[:, icol : icol + nsub * 8],
                                        num_idxs=nidx,
                                        num_idxs_reg=nidx,
                                        elem_size=din,
                                        queue_num=self_qn[0] % N_SWDGE_QUEUES,
                                    )
                                    self_qn[0] += 1
                                    calls.append((et, nsub))
                                    icol += nsub * 8
                                    done += nsub
                        bsel = bpool.tile([P, ncht_t * P], BF16, tag="bsel")
                        nc.vector.tensor_tensor(
                            out=bsel[:].rearrange("p (s j) -> p s j", j=P),
                            in0=iota_sb[:, 0 : ncht_t * P].rearrange(
                                "p (s j) -> p s j", j=P
                            ),
                            in1=dloc_sb[:, dcol : dcol + ncht_t, None].broadcast_to(
                                [P, ncht_t, P]
                            ),
                            op=mybir.AluOpType.is_equal,
                        )
                        pre[t] = (xo, calls, bsel, ncht_t)

                    # -- phase 2: scatter-add matmuls per tile
                    h_tiles = []
                    for t in tlist:
                        xo, calls, bsel, ncht_t = pre[t]
                        aggps = agg_ps.tile([P, din], F32, tag="agg")
                        k = 0
                        for et, nsub in calls:
                            for kk in range(nsub):
                                nc.tensor.matmul(
                                    aggps[:],
                                    lhsT=bsel[:, k * P : (k + 1) * P],
                                    rhs=et[:, kk * din : (kk + 1) * din],
                                    start=(k == 0),
                                    stop=False,
                                )
                                k += 1
                        nc.tensor.matmul(
                            aggps[:], lhsT=id16_sb[:], rhs=xo[:], start=False, stop=True
                        )
                        h_sb = hpool.tile([P, din], F32, tag="h")
                        nc.vector.tensor_copy(h_sb[:], aggps[:])
                        h_tiles.append(h_sb)

                    # transpose h -> feature-major [din, nodes_c]
                    hfm = fpool.tile([P, fch * 512], F32, tag="hfm")
                    for ti, t in enumerate(tlist):
                        tps = tp_ps.tile([P, fch * P], F32, tag="tp")
                        for f in range(fch):
                            nc.tensor.transpose(
                                out=tps[:, f * P : (f + 1) * P],
                                in_=h_tiles[ti][:, f * P : (f + 1) * P],
                                identity=idf_sb[:],
                            )
                        for f in range(fch):
                            nc.vector.tensor_copy(
                                hfm[:, f * 512 + ti * P : f * 512 + (ti + 1) * P],
                                tps[:, f * P : (f + 1) * P],
                            )

                    # MLP1: z = relu(h @ W1T + b1), feature-major
                    z_tiles = []
                    for fo in range(4):
                        zps = z_ps.tile([P, 512], F32, tag="z")
                        for fi in range(fch):
                            nc.tensor.matmul(
                                zps[:, :nodes_c],
                                lhsT=w1t_sb[:, fi * H + fo * P : fi * H + (fo + 1) * P],
                                rhs=hfm[:, fi * 512 : fi * 512 + nodes_c],
                                start=(fi == 0),
                                stop=(fi == fch - 1),
                            )
                        z_sb = zpool.tile([P, 512], F32, tag="z_sb")
                        nc.scalar.activation(
                            z_sb[:, :nodes_c],
                            zps[:, :nodes_c],
                            mybir.ActivationFunctionType.Relu,
                            bias=b1_sb[:, lay * 4 + fo : lay * 4 + fo + 1],
                        )
                        z_tiles.append(z_sb)

                    # MLP2: h_next = z @ W2T + b2, node-major
                    for ti, t in enumerate(tlist):
                        rows = tile_rows(t)
                        h2ps = h2_ps.tile([P, H], F32, tag="h2")
                        for zf in range(4):
                            nc.tensor.matmul(
                                h2ps[:rows, :],
                                lhsT=z_tiles[zf][:, ti * P : ti * P + rows],
                                rhs=w2t_sb[:, zf * H : (zf + 1) * H],
                                start=(zf == 0),
                                stop=(zf == 3),
                            )
                        if lay < N_LAYERS - 1:
                            agt = agpool.tile([P, H], BF16, tag="ag")
                            nc.vector.tensor_tensor(
                                out=agt[:rows, :],
                                in0=h2ps[:rows, :],
                                in1=b2_sb[:rows, :],
                                op=mybir.AluOpType.add,
                            )
                            bt = next(
                                bi for bi, (s0, e0) in enumerate(BANKS)
                                if s0 <= t * P < e0
                            )
                            o = t * P - BANKS[bt][0]
                            nc.sync.dma_start(
                                agx[lay % 2][bt][o : o + rows, :], agt[:rows, :]
                            )
                        else:
                            hn = agpool.tile([P, H], BF16, tag="hn")
                            nc.vector.tensor_tensor(
                                out=hn[:rows, :],
                                in0=h2ps[:rows, :],
                                in1=b2_sb[:rows, :],
                                op=mybir.AluOpType.add,
                            )
                            nc.tensor.matmul(
                                poolps[:],
                                lhsT=pool_sb[:rows, t * G : (t + 1) * G],
                                rhs=hn[:rows, :],
                                start=(t == 0),
                                stop=(t == TILES - 1),
                            )

                    # split AllGather: each bank fires as soon as its tiles are done
                    if lay < N_LAYERS - 1:
                        for b in range(NBANKS):
                            bank_done = cdiv(BANKS[b][1], P) - 1
                            if bank_done not in tlist:
                                continue
                            agt_, xft_ = agx[lay % 2][b], xfx[lay % 2][b]
                            if _no_cc():
                                nc.sync.dma_start(
                                    xft_[0 : agt_.shape[0], :], agt_[:, :]
                                )
                            else:
                                nc.gpsimd.collective_compute(
                                    "AllGather",
                                    mybir.AluOpType.bypass,
                                    replica_groups=rg,
                                    ins=[agt_[:, :]],
                                    outs=[xft_[:, :]],
                                )

            # ---- pooled epilogue (replicated on every core)
            poolsb = cpool.tile([G, H], F32)
            nc.vector.tensor_copy(poolsb[:], poolps[:])
            nc.sync.dma_start(prb[:, :], poolsb[:])
            if _no_cc():
                nc.sync.dma_start(pro[:, :], prb[:, :])
            else:
                nc.gpsimd.collective_compute(
                    "AllReduce",
                    mybir.AluOpType.add,
                    replica_groups=rg,
                    ins=[prb[:, :]],
                    outs=[pro[:, :]],
                )
            pr_sb = cpool.tile([G, H], F32)
            nc.sync.dma_start(pr_sb[:], pro[:, :])
            nc.vector.tensor_scalar_mul(pr_sb[:], pr_sb[:], cinv_sb[:, 0:1])
            tmp = cpool.tile([G, H], F32)
            nc.vector.tensor_tensor(
                out=tmp[:], in0=pr_sb[:], in1=fcw_sb[:], op=mybir.AluOpType.mult
            )
            dot = cpool.tile([G, 1], F32)
            nc.vector.tensor_reduce(
                out=dot[:], in_=tmp[:], axis=mybir.AxisListType.X, op=mybir.AluOpType.add
            )
            osb = cpool.tile([G, 1], F32)
            nc.scalar.activation(
                osb[:],
                dot[:],
                mybir.ActivationFunctionType.Sigmoid,
                bias=fcb_sb[:, 0:1],
            )
            nc.sync.dma_start(out_ext[:, :], osb[:])

    nc.compile()
    return nc


# ---------------- host wrapper ------------------------------------------------
def _prepare_inputs(x, edge_index, batch, w1_0, b1_0, w2_0, b2_0,
                    w1_rest, b1_rest, w2_rest, b2_rest, fc_w, fc_b):
    x0 = np.asarray(x, np.float32).astype(nbf16)
    nch, idx16, dlocs, x1g = _preprocess_edges(np.asarray(edge_index), x0)
    pool, cinv = _build_pool_onehot(batch)
    nch_max = int(nch.sum(axis=1).max())

    w1tl = [_spectral_normalize(w1_0).T]
    w2tl = [_spectral_normalize(w2_0).T]
    b1l = [np.asarray(b1_0, np.float32)]
    b2l = [np.asarray(b2_0, np.float32)]
    for i in range(N_LAYERS - 1):
        w1tl.append(_spectral_normalize(w1_rest[i]).T)
        w2tl.append(_spectral_normalize(w2_rest[i]).T)
        b1l.append(np.asarray(b1_rest[i], np.float32))
        b2l.append(np.asarray(b2_rest[i], np.float32))

    w1t0_np = np.ascontiguousarray(w1tl[0])                      # [128, 512]
    w1tr_np = np.ascontiguousarray(np.concatenate(w1tl[1:], 0))  # [3*512, 512]
    w2t_np = np.ascontiguousarray(np.concatenate(w2tl, 0))       # [4*512, 512]
    b1c_np = np.zeros((P, N_LAYERS * 4), np.float32)
    for l in range(N_LAYERS):
        for f in range(4):
            b1c_np[:, l * 4 + f] = b1l[l][f * P : (f + 1) * P]
    b2bc_np = np.zeros((N_LAYERS * P, H), np.float32)
    for l in range(N_LAYERS):
        b2bc_np[l * P : (l + 1) * P, :] = b2l[l][None, :]

    iota_np = np.tile(np.arange(P, dtype=np.float32), nch_max)[None, :].repeat(P, 0)
    shared = {
        "w1t0": w1t0_np,
        "w1tr": w1tr_np,
        "w2t": w2t_np,
        "b1c": b1c_np,
        "b2bc": b2bc_np,
        "iotar": iota_np.astype(nbf16),
        "ident16": np.eye(P, dtype=np.float32).astype(nbf16),
        "identf": np.eye(P, dtype=np.float32),
        "cinv": cinv[:, None],
        "fcwb": np.repeat(np.asarray(fc_w, np.float32), G, axis=0),
        "fcb": np.full((G, 1), np.float32(np.asarray(fc_b).reshape(-1)[0]), np.float32),
    }
    in_maps = []
    for c in range(CORES):
        m = dict(shared)
        m["xown0"] = np.ascontiguousarray(x0[c * NPC : (c + 1) * NPC])
        m["x1g"] = np.ascontiguousarray(x1g[c])
        m["idx16"] = np.ascontiguousarray(idx16[c])
        m["dlocs"] = np.ascontiguousarray(dlocs[c])
        m["pool1h"] = np.ascontiguousarray(pool[c]).astype(nbf16)
        in_maps.append(m)
    return nch, in_maps


_prog_cache = {}
last_results = None


def kernel(x, edge_index, batch, w1_0, b1_0, w2_0, b2_0,
           w1_rest, b1_rest, w2_rest, b2_rest, fc_w, fc_b, **run_kwargs):
    global last_results
    nch, in_maps = _prepare_inputs(
        x, edge_index, batch, w1_0, b1_0, w2_0, b2_0,
        w1_rest, b1_rest, w2_rest, b2_rest, fc_w, fc_b,
    )
    key = nch.tobytes()
    if key not in _prog_cache:
        _prog_cache[key] = build_program(nch)
    nc = _prog_cache[key]
    res = run_bass_kernel_spmd(nc, in_maps, core_ids=list(range(CORES)), **run_kwargs)
    last_results = res
    return np.asarray(res.results[0]["out"], np.float32)
